# revision 41
# baseline (speedup 1.0000x reference)
"""Bahdanau additive attention on 8 Trainium2 NeuronCores.

reference:
    q_proj = (query @ Wa_w.T + Wa_b)[:, None, :]          # [B, 1, H]
    k_proj = einsum('bsh,dh->bsd', keys, Ua_w) + Ua_b     # [B, S, H]
    scores = einsum('bsh,h->bs', tanh(q_proj + k_proj), Va_w[0]) + Va_b[0]
    attn   = softmax(scores, axis=1)                      # [B, S]
    context= einsum('bs,bsh->bh', attn, keys)             # [B, H]

Sharding: pure data parallel over B (64 -> 8 per core); weights replicated.

Device pipeline (per core, B_loc=8, S=2048, H=D=512), all matmuls bf16:
  - keysT bf16 (h on partitions) drives k_projT = Ua_w @ k^T in PSUM;
    tanh + per-partition q_proj bias fused on ScalarE.
  - scores row via M=1 matmuls (lhsT = Va column) accumulated in PSUM.
  - p = exp(scores + Va_b) on ScalarE with fused running sum (accum_out).
    No max-subtraction: |scores| <= sum|Va| + |Va_b| ~ 23 fits fp32 exp.
  - context rides VectorE: one K=1 matmul broadcasts the p chunk across
    partitions, then scalar_tensor_tensor (float scalar + fused accum_out)
    does multiply+free-reduce against the resident keysT tiles in a single
    DVE op per h-tile — no second keys copy, no p transposes.
  - per batch the four [128,1] context columns are transposed back to one
    [1,512] PSUM row with identity matmuls and scaled by 1/l in one op.
  - Three-stage software pipeline over (batch, s-chunk) groups g:
    projection matmuls of g, score matmuls of g-1, context of g-2 — every
    PE instruction's ACT/DVE dependency is a full chunk old, so the PE FIFO
    streams at the N=512 back-to-back rate (~216 ns/matmul).
"""

import numpy as np
import ml_dtypes

B, S, H = 64, 2048, 512
NCORES = 8
BL = B // NCORES          # batches per core
HC = H // 128             # h chunks (contraction)
DT = H // 128             # d tiles (partition dim of k_projT)
SCW = 512                 # s-chunk width (PSUM bank limit)
SC = S // SCW             # s chunks
ST = S // 128             # s tiles (for p^T / context)

_cache = {}


def _build(do_compile=True):
    import concourse.bass as bass
    import concourse.tile as tile
    from concourse import bacc, mybir

    f32 = mybir.dt.float32
    bf16 = mybir.dt.bfloat16
    AF = mybir.ActivationFunctionType
    ALU = mybir.AluOpType
    AX = mybir.AxisListType

    nc = bacc.Bacc("TRN2", target_bir_lowering=False, debug=False)

    keysT = nc.declare_dram_parameter("keysT", [BL, H, S], bf16, isOutput=False)
    uapack = nc.declare_dram_parameter(
        "uapack", [128, HC, H], bf16, isOutput=False
    )
    wqpack = nc.declare_dram_parameter(
        "wqpack", [128, HC, H + BL], bf16, isOutput=False
    )
    bias_wu = nc.declare_dram_parameter("bias_wu", [128, DT], f32, isOutput=False)
    va = nc.declare_dram_parameter("va", [128, DT], bf16, isOutput=False)
    vab = nc.declare_dram_parameter("vab", [1, 1], f32, isOutput=False)
    ident = nc.declare_dram_parameter("ident", [128, 128], f32, isOutput=False)
    out_ctx = nc.declare_dram_parameter("out_ctx", [BL, H], f32, isOutput=True)
    out_attn = nc.declare_dram_parameter("out_attn", [BL, S], f32, isOutput=True)

    with tile.TileContext(nc) as tc:
        with (
            tc.tile_pool(name="persist", bufs=1) as pp,
            tc.tile_pool(name="keys", bufs=2) as kp,
            tc.tile_pool(name="work", bufs=10) as wp,
            tc.tile_pool(name="rows", bufs=3) as rp,
            tc.tile_pool(name="outs", bufs=2) as op,
            tc.tile_pool(name="ps", bufs=1, space="PSUM") as ps,
        ):
            ua_sb = pp.tile([128, HC, H], bf16, tag="uapack")
            UaT_sb = [ua_sb[:, hc, :] for hc in range(HC)]
            bwu_sb = pp.tile([128, DT], f32, tag="bwu")
            nc.gpsimd.dma_start(bwu_sb[:], bias_wu[:])
            va_sb = pp.tile([128, DT], bf16, tag="va")
            nc.gpsimd.dma_start(va_sb[:], va[:])
            vab_sb = pp.tile([1, 1], f32, tag="vab")
            nc.gpsimd.dma_start(vab_sb[:], vab[:])
            ones_bf = pp.tile([1, 128], bf16, tag="ones_bf")
            nc.vector.memset(ones_bf[:], 1.0)
            ident_sb = pp.tile([128, 128], f32, tag="ident")
            nc.gpsimd.dma_start(ident_sb[:], ident[:])

            qb_sb = []

            def q_proj():
                for dt in range(DT):
                    pq = ps.tile([128, BL], f32, tag="pmisc", name="pq", bufs=1)
                    dsl = slice(128 * dt, 128 * (dt + 1))
                    for hc in range(HC):
                        nc.tensor.matmul(
                            pq[:], wq_sb[:, hc, dsl], wq_sb[:, hc, H : H + BL],
                            start=(hc == 0), stop=(hc == HC - 1),
                        )
                    qb = pp.tile([128, BL], f32, tag=f"qb{dt}", name=f"qb{dt}")
                    nc.scalar.activation(
                        qb[:], pq[:], AF.Identity, bias=bwu_sb[:, dt : dt + 1]
                    )
                    qb_sb.append(qb)

            l_parts = pp.tile([1, BL * SC], f32, tag="l_parts")

            # per-batch live state
            kT = [None] * BL
            p_row = [None] * BL

            wq_sb = pp.tile([128, HC, H + BL], bf16, tag="wqpack")

            def load_batch(b, interleave_ua=False):
                kT[b] = []
                if interleave_ua:
                    nc.sync.dma_start(ua_sb[:], uapack[:])
                for ht in range(HC):
                    t = kp.tile([128, S], bf16, tag=f"kT{ht}", name=f"kT{ht}_{b}", bufs=3)
                    nc.sync.dma_start(
                        t[:], keysT[b, 128 * ht : 128 * (ht + 1), :]
                    )
                    kT[b].append(t)
                    if interleave_ua and ht == 0:
                        nc.sync.dma_start(wq_sb[:], wqpack[:])
                p_row[b] = rp.tile([1, S], bf16, tag="p_row", name=f"p_row{b}")

            state = {}

            def proj_chunk(b, sc, mid_hook=None, startup=False):
                """16 projection MMs + 4 tanh; score MMs run one chunk later.

                startup: h-chunk-major order over the first 3 banks so the
                first matmuls only need the first keysT tile's DMA."""
                ssl = slice(SCW * sc, SCW * (sc + 1))
                pks = [
                    ps.tile([128, SCW], f32, tag="pk", name="pk", bufs=3)
                    for _ in range(DT)
                ]
                if startup:
                    order = [(hc, dt) for hc in range(HC) for dt in range(DT - 1)]
                    order += [(hc, DT - 1) for hc in range(HC)]
                else:
                    order = [(hc, dt) for dt in range(DT) for hc in range(HC)]
                for hc, dt in order:
                    dsl = slice(128 * dt, 128 * (dt + 1))
                    nc.tensor.matmul(
                        pks[dt][:], UaT_sb[hc][:, dsl], kT[b][hc][:, ssl],
                        start=(hc == 0), stop=(hc == HC - 1),
                    )
                if mid_hook is not None:
                    mid_hook()
                t_tiles = []
                for dt in range(DT):
                    t_sb = wp.tile([128, SCW], bf16, tag="t", name="t")
                    nc.scalar.activation(
                        t_sb[:], pks[dt][:], AF.Tanh, bias=qb_sb[dt][:, b : b + 1]
                    )
                    t_tiles.append(t_sb)
                state[(b, sc)] = t_tiles

            def score_chunk(b, sc):
                """4 score MMs (tanh results are a chunk old -> no PE stall),
                then exp with fused partial sum."""
                ssl = slice(SCW * sc, SCW * (sc + 1))
                t_tiles = state.pop((b, sc))
                psc = ps.tile([1, SCW], f32, tag="psc", name=f"psc{b}_{sc}", bufs=2)
                for dt in range(DT):
                    nc.tensor.matmul(
                        psc[:], va_sb[:, dt : dt + 1], t_tiles[dt][:],
                        start=(dt == 0), stop=(dt == DT - 1),
                    )
                col = SC * b + sc
                nc.scalar.activation(
                    p_row[b][0:1, ssl], psc[:], AF.Exp, bias=vab_sb[0:1, 0:1],
                    accum_out=l_parts[0:1, col : col + 1],
                )

            ctx_parts = [
                pp.tile([128, BL * SC], f32, tag=f"ctxp{ht}", name=f"ctxp{ht}")
                for ht in range(HC)
            ]

            def ctx_chunk(b, sc):
                """Broadcast the p chunk across partitions with one K=1 matmul,
                then fused multiply+free-reduce on VectorE per keysT h-tile:
                ctx_parts[ht][:, col] = sum_s kT[ht][:, s] * p[s]."""
                ssl = slice(SCW * sc, SCW * (sc + 1))
                col = SC * b + sc
                pb = ps.tile([128, SCW], f32, tag="pb", name="pb", bufs=2)
                nc.tensor.matmul(
                    pb[:], ones_bf[:], p_row[b][0:1, ssl], start=True, stop=True
                )
                for ht in range(HC):
                    scr = wp.tile([128, SCW], bf16, tag="scr", name="scr", bufs=4)
                    nc.vector.scalar_tensor_tensor(
                        scr[:], kT[b][ht][:, ssl], 1.0, pb[:],
                        ALU.mult, ALU.mult,
                        accum_out=ctx_parts[ht][:, col : col + 1],
                    )

            def finish_batch(b):
                """softmax normalization + outputs for batch b."""
                lsum = rp.tile([1, 1], f32, tag="lsum", name=f"lsum{b}")
                nc.vector.tensor_reduce(
                    lsum[:], l_parts[0:1, SC * b : SC * (b + 1)], AX.X, ALU.add
                )
                linv_b = rp.tile([1, 1], f32, tag="linv", name=f"linv{b}")
                nc.vector.reciprocal(linv_b[:], lsum[:])
                attn_row = op.tile([1, S], f32, tag="attn_row", name="attn_row")
                nc.vector.tensor_scalar_mul(attn_row[:], p_row[b][:], linv_b[:])
                nc.gpsimd.dma_start(out_attn[b : b + 1, :], attn_row[:])

                # transpose the 4 context columns back to one [1, 512] row
                # via K=128/M=1/N=128 identity matmuls, then scale by 1/l
                pcr = ps.tile([1, H], f32, tag="pmisc", name=f"pcr{b}", bufs=1)
                for ht in range(HC):
                    csum = rp.tile([128, 1], f32, tag="csum", name=f"csum{b}_{ht}")
                    nc.vector.tensor_reduce(
                        csum[:], ctx_parts[ht][:, SC * b : SC * (b + 1)],
                        AX.X, ALU.add,
                    )
                    nc.tensor.matmul(
                        pcr[0:1, 128 * ht : 128 * (ht + 1)], csum[:], ident_sb[:],
                        start=True, stop=True,
                    )
                ctx_row = op.tile([1, H], f32, tag="ctx_row", name="ctx_row")
                nc.vector.tensor_scalar_mul(ctx_row[:], pcr[:], linv_b[:])
                nc.gpsimd.dma_start(out_ctx[b : b + 1, :], ctx_row[:])

            # ---- software-pipelined emission over (batch, chunk) ----
            load_batch(0, interleave_ua=True)
            chunks = [(b, sc) for b in range(BL) for sc in range(SC)]
            N = len(chunks)
            for g in range(N + 2):
                if g < N:
                    b, sc = chunks[g]
                    if sc == 0 and b + 1 < BL:
                        load_batch(b + 1)   # prefetch next batch's keys
                    proj_chunk(b, sc, mid_hook=(q_proj if g == 0 else None), startup=(g == 0))
                if 0 <= g - 1 < N:
                    score_chunk(*chunks[g - 1])
                if 0 <= g - 2 < N:
                    tb, tsc = chunks[g - 2]
                    ctx_chunk(tb, tsc)
                    if tsc == SC - 1:
                        finish_batch(tb)

    if do_compile:
        nc.compile()
    return nc


def _prep_in_maps(query, keys, Wa_w, Wa_b, Ua_w, Ua_b, Va_w, Va_b):
    bf16 = ml_dtypes.bfloat16
    keysT = np.ascontiguousarray(keys.astype(bf16).transpose(0, 2, 1))  # [B,H,S]
    queryT = query.T.astype(bf16)                               # [H, B]
    WaT = Wa_w.T.astype(bf16)
    UaT = Ua_w.T.astype(bf16)
    bias_wu = np.ascontiguousarray(
        (Wa_b + Ua_b).astype(np.float32).reshape(DT, 128).T
    )
    va_f = np.ascontiguousarray(Va_w[0].astype(bf16).reshape(DT, 128).T)
    vab = Va_b.reshape(1, 1).astype(np.float32)
    ident = np.eye(128, dtype=np.float32)

    in_maps = []
    for c in range(NCORES):
        bs = slice(BL * c, BL * (c + 1))
        uapk = np.ascontiguousarray(
            UaT.reshape(HC, 128, H).transpose(1, 0, 2)
        )
        wqpk = np.concatenate([WaT, queryT[:, bs]], axis=1).reshape(
            HC, 128, H + BL
        )
        wqpk = np.ascontiguousarray(wqpk.transpose(1, 0, 2))
        in_maps.append(
            {
                "keysT": np.ascontiguousarray(keysT[bs]),
                "uapack": uapk,
                "wqpack": wqpk,
                "bias_wu": bias_wu,
                "va": va_f,
                "vab": vab,
                "ident": ident,
            }
        )
    return in_maps


def kernel(query, keys, Wa_w, Wa_b, Ua_w, Ua_b, Va_w, Va_b):
    from concourse.bass_utils import run_bass_kernel_spmd

    query, keys = np.asarray(query), np.asarray(keys)
    Wa_w, Wa_b = np.asarray(Wa_w), np.asarray(Wa_b)
    Ua_w, Ua_b = np.asarray(Ua_w), np.asarray(Ua_b)
    Va_w, Va_b = np.asarray(Va_w), np.asarray(Va_b)
    if "nc" not in _cache:
        _cache["nc"] = _build()
    nc = _cache["nc"]

    in_maps = _prep_in_maps(query, keys, Wa_w, Wa_b, Ua_w, Ua_b, Va_w, Va_b)
    res = run_bass_kernel_spmd(nc, in_maps, core_ids=list(range(NCORES)))
    context = np.concatenate(
        [res.results[c]["out_ctx"] for c in range(NCORES)], axis=0
    )
    attn = np.concatenate(
        [res.results[c]["out_attn"] for c in range(NCORES)], axis=0
    )
    return context, attn


# revision 42
# speedup vs baseline: 1.1948x; 1.1948x over previous
"""Bahdanau additive attention on 8 Trainium2 NeuronCores.

reference:
    q_proj = (query @ Wa_w.T + Wa_b)[:, None, :]          # [B, 1, H]
    k_proj = einsum('bsh,dh->bsd', keys, Ua_w) + Ua_b     # [B, S, H]
    scores = einsum('bsh,h->bs', tanh(q_proj + k_proj), Va_w[0]) + Va_b[0]
    attn   = softmax(scores, axis=1)                      # [B, S]
    context= einsum('bs,bsh->bh', attn, keys)             # [B, H]

Sharding: pure data parallel over B (64 -> 8 per core); weights replicated.

Device pipeline (per core, B_loc=8, S=2048, H=D=512), all matmuls bf16:
  - keysT bf16 (h on partitions) drives k_projT = Ua_w @ k^T in PSUM;
    tanh + per-partition q_proj bias fused on ScalarE.
  - scores row via M=1 matmuls (lhsT = Va column) accumulated in PSUM.
  - p = exp(scores + Va_b) on ScalarE with fused running sum (accum_out).
    No max-subtraction: |scores| <= sum|Va| + |Va_b| ~ 23 fits fp32 exp.
  - context rides VectorE: one K=1 matmul broadcasts the p chunk across
    partitions, then scalar_tensor_tensor (float scalar + fused accum_out)
    does multiply+free-reduce against the resident keysT tiles in a single
    DVE op per h-tile — no second keys copy, no p transposes.
  - per batch the four [128,1] context columns are transposed back to one
    [1,512] PSUM row with identity matmuls and scaled by 1/l in one op.
  - Three-stage software pipeline over (batch, s-chunk) groups g:
    projection matmuls of g, score matmuls of g-1, context of g-2 — every
    PE instruction's ACT/DVE dependency is a full chunk old, so the PE FIFO
    streams at the N=512 back-to-back rate (~216 ns/matmul).
"""

import numpy as np
import ml_dtypes

B, S, H = 64, 2048, 512
NCORES = 8
BL = B // NCORES          # batches per core
HC = H // 128             # h chunks (contraction)
DT = H // 128             # d tiles (partition dim of k_projT)
SCW = 512                 # s-chunk width (PSUM bank limit)
SC = S // SCW             # s chunks
ST = S // 128             # s tiles (for p^T / context)

_cache = {}


def _build(do_compile=True):
    import concourse.bass as bass
    import concourse.tile as tile
    from concourse import bacc, mybir

    f32 = mybir.dt.float32
    bf16 = mybir.dt.bfloat16
    AF = mybir.ActivationFunctionType
    ALU = mybir.AluOpType
    AX = mybir.AxisListType

    nc = bacc.Bacc("TRN2", target_bir_lowering=False, debug=False)

    keysT = nc.declare_dram_parameter("keysT", [BL, H, S], bf16, isOutput=False)
    uapack = nc.declare_dram_parameter(
        "uapack", [128, HC, H], bf16, isOutput=False
    )
    wqpack = nc.declare_dram_parameter(
        "wqpack", [128, HC, H + BL], bf16, isOutput=False
    )
    bias_wu = nc.declare_dram_parameter("bias_wu", [128, DT], f32, isOutput=False)
    va = nc.declare_dram_parameter("va", [128, DT], bf16, isOutput=False)
    vab = nc.declare_dram_parameter("vab", [1, 1], f32, isOutput=False)
    ident = nc.declare_dram_parameter("ident", [128, 128], f32, isOutput=False)
    out_ctx = nc.declare_dram_parameter("out_ctx", [BL, H], f32, isOutput=True)
    out_attn = nc.declare_dram_parameter("out_attn", [BL, S], f32, isOutput=True)

    with tile.TileContext(nc) as tc:
        with (
            tc.tile_pool(name="persist", bufs=1) as pp,
            tc.tile_pool(name="keys", bufs=2) as kp,
            tc.tile_pool(name="work", bufs=10) as wp,
            tc.tile_pool(name="rows", bufs=3) as rp,
            tc.tile_pool(name="outs", bufs=2) as op,
            tc.tile_pool(name="ps", bufs=1, space="PSUM") as ps,
        ):
            ua_sb = pp.tile([128, HC, H], bf16, tag="uapack")
            UaT_sb = [ua_sb[:, hc, :] for hc in range(HC)]
            bwu_sb = pp.tile([128, DT], f32, tag="bwu")
            nc.gpsimd.dma_start(bwu_sb[:], bias_wu[:])
            va_sb = pp.tile([128, DT], bf16, tag="va")
            nc.gpsimd.dma_start(va_sb[:], va[:])
            vab_sb = pp.tile([1, 1], f32, tag="vab")
            nc.gpsimd.dma_start(vab_sb[:], vab[:])
            ones_bf = pp.tile([1, 128], bf16, tag="ones_bf")
            nc.vector.memset(ones_bf[:], 1.0)
            ident_sb = pp.tile([128, 128], f32, tag="ident")
            nc.gpsimd.dma_start(ident_sb[:], ident[:])

            qb_sb = []

            def q_proj():
                for dt in range(DT):
                    pq = ps.tile([128, BL], f32, tag="pmisc", name="pq", bufs=1)
                    dsl = slice(128 * dt, 128 * (dt + 1))
                    for hc in range(HC):
                        nc.tensor.matmul(
                            pq[:], wq_sb[:, hc, dsl], wq_sb[:, hc, H : H + BL],
                            start=(hc == 0), stop=(hc == HC - 1),
                        )
                    qb = pp.tile([128, BL], f32, tag=f"qb{dt}", name=f"qb{dt}")
                    nc.scalar.activation(
                        qb[:], pq[:], AF.Identity, bias=bwu_sb[:, dt : dt + 1]
                    )
                    qb_sb.append(qb)

            l_parts = pp.tile([1, BL * SC], f32, tag="l_parts")

            # per-batch live state
            kT = [None] * BL
            p_row = [None] * BL

            wq_sb = pp.tile([128, HC, H + BL], bf16, tag="wqpack")

            def load_batch(b, interleave_ua=False):
                kT[b] = []
                if interleave_ua:
                    nc.sync.dma_start(ua_sb[:], uapack[:])
                for ht in range(HC):
                    t = kp.tile([128, S], bf16, tag=f"kT{ht}", name=f"kT{ht}_{b}", bufs=3)
                    nc.sync.dma_start(
                        t[:], keysT[b, 128 * ht : 128 * (ht + 1), :]
                    )
                    kT[b].append(t)
                    if interleave_ua and ht == 0:
                        nc.sync.dma_start(wq_sb[:], wqpack[:])
                p_row[b] = rp.tile([1, S], bf16, tag="p_row", name=f"p_row{b}")

            state = {}

            def proj_chunk(b, sc, mid_hook=None, startup=False):
                """16 projection MMs + 4 tanh; score MMs run one chunk later.

                startup: h-chunk-major order over the first 3 banks so the
                first matmuls only need the first keysT tile's DMA."""
                ssl = slice(SCW * sc, SCW * (sc + 1))
                pks = [
                    ps.tile([128, SCW], f32, tag="pk", name="pk", bufs=3)
                    for _ in range(DT)
                ]
                order = [(hc, dt) for dt in range(DT) for hc in range(HC)]
                for hc, dt in order:
                    dsl = slice(128 * dt, 128 * (dt + 1))
                    nc.tensor.matmul(
                        pks[dt][:], UaT_sb[hc][:, dsl], kT[b][hc][:, ssl],
                        start=(hc == 0), stop=(hc == HC - 1),
                    )
                if mid_hook is not None:
                    mid_hook()
                t_tiles = []
                for dt in range(DT):
                    t_sb = wp.tile([128, SCW], bf16, tag="t", name="t")
                    nc.scalar.activation(
                        t_sb[:], pks[dt][:], AF.Tanh, bias=qb_sb[dt][:, b : b + 1]
                    )
                    t_tiles.append(t_sb)
                state[(b, sc)] = t_tiles

            def score_chunk(b, sc):
                """4 score MMs (tanh results are a chunk old -> no PE stall),
                then exp with fused partial sum."""
                ssl = slice(SCW * sc, SCW * (sc + 1))
                t_tiles = state.pop((b, sc))
                psc = ps.tile([1, SCW], f32, tag="psc", name=f"psc{b}_{sc}", bufs=2)
                for dt in range(DT):
                    nc.tensor.matmul(
                        psc[:], va_sb[:, dt : dt + 1], t_tiles[dt][:],
                        start=(dt == 0), stop=(dt == DT - 1),
                    )
                col = SC * b + sc
                nc.scalar.activation(
                    p_row[b][0:1, ssl], psc[:], AF.Exp, bias=vab_sb[0:1, 0:1],
                    accum_out=l_parts[0:1, col : col + 1],
                )

            ctx_parts = [
                pp.tile([128, BL * SC], f32, tag=f"ctxp{ht}", name=f"ctxp{ht}")
                for ht in range(HC)
            ]

            def ctx_chunk(b, sc):
                """Broadcast the p chunk across partitions with one K=1 matmul,
                then fused multiply+free-reduce on VectorE per keysT h-tile:
                ctx_parts[ht][:, col] = sum_s kT[ht][:, s] * p[s]."""
                ssl = slice(SCW * sc, SCW * (sc + 1))
                col = SC * b + sc
                pb = ps.tile([128, SCW], f32, tag="pb", name="pb", bufs=2)
                nc.tensor.matmul(
                    pb[:], ones_bf[:], p_row[b][0:1, ssl], start=True, stop=True
                )
                for ht in range(HC):
                    scr = wp.tile([128, SCW], bf16, tag="scr", name="scr", bufs=4)
                    nc.vector.scalar_tensor_tensor(
                        scr[:], kT[b][ht][:, ssl], 1.0, pb[:],
                        ALU.mult, ALU.mult,
                        accum_out=ctx_parts[ht][:, col : col + 1],
                    )

            def finish_batch(b):
                """softmax normalization + outputs for batch b."""
                lsum = rp.tile([1, 1], f32, tag="lsum", name=f"lsum{b}")
                nc.vector.tensor_reduce(
                    lsum[:], l_parts[0:1, SC * b : SC * (b + 1)], AX.X, ALU.add
                )
                linv_b = rp.tile([1, 1], f32, tag="linv", name=f"linv{b}")
                nc.vector.reciprocal(linv_b[:], lsum[:])
                attn_row = op.tile([1, S], f32, tag="attn_row", name="attn_row")
                nc.vector.tensor_scalar_mul(attn_row[:], p_row[b][:], linv_b[:])
                nc.gpsimd.dma_start(out_attn[b : b + 1, :], attn_row[:])

                # transpose the 4 context columns back to one [1, 512] row
                # via K=128/M=1/N=128 identity matmuls, then scale by 1/l
                pcr = ps.tile([1, H], f32, tag="pmisc", name=f"pcr{b}", bufs=1)
                for ht in range(HC):
                    csum = rp.tile([128, 1], f32, tag="csum", name=f"csum{b}_{ht}")
                    nc.vector.tensor_reduce(
                        csum[:], ctx_parts[ht][:, SC * b : SC * (b + 1)],
                        AX.X, ALU.add,
                    )
                    nc.tensor.matmul(
                        pcr[0:1, 128 * ht : 128 * (ht + 1)], csum[:], ident_sb[:],
                        start=True, stop=True,
                    )
                ctx_row = op.tile([1, H], f32, tag="ctx_row", name="ctx_row")
                nc.vector.tensor_scalar_mul(ctx_row[:], pcr[:], linv_b[:])
                nc.gpsimd.dma_start(out_ctx[b : b + 1, :], ctx_row[:])

            # ---- software-pipelined emission over (batch, chunk) ----
            load_batch(0, interleave_ua=True)
            chunks = [(b, sc) for b in range(BL) for sc in range(SC)]
            N = len(chunks)
            for g in range(N + 2):
                if g < N:
                    b, sc = chunks[g]
                    if sc == 0 and b + 1 < BL:
                        load_batch(b + 1)   # prefetch next batch's keys
                    proj_chunk(b, sc, mid_hook=(q_proj if g == 0 else None), startup=(g == 0))
                if 0 <= g - 1 < N:
                    score_chunk(*chunks[g - 1])
                if 0 <= g - 2 < N:
                    tb, tsc = chunks[g - 2]
                    ctx_chunk(tb, tsc)
                    if tsc == SC - 1:
                        finish_batch(tb)

    if do_compile:
        nc.compile()
    return nc


def _prep_in_maps(query, keys, Wa_w, Wa_b, Ua_w, Ua_b, Va_w, Va_b):
    bf16 = ml_dtypes.bfloat16
    keysT = np.ascontiguousarray(keys.astype(bf16).transpose(0, 2, 1))  # [B,H,S]
    queryT = query.T.astype(bf16)                               # [H, B]
    WaT = Wa_w.T.astype(bf16)
    UaT = Ua_w.T.astype(bf16)
    bias_wu = np.ascontiguousarray(
        (Wa_b + Ua_b).astype(np.float32).reshape(DT, 128).T
    )
    va_f = np.ascontiguousarray(Va_w[0].astype(bf16).reshape(DT, 128).T)
    vab = Va_b.reshape(1, 1).astype(np.float32)
    ident = np.eye(128, dtype=np.float32)

    in_maps = []
    for c in range(NCORES):
        bs = slice(BL * c, BL * (c + 1))
        uapk = np.ascontiguousarray(
            UaT.reshape(HC, 128, H).transpose(1, 0, 2)
        )
        wqpk = np.concatenate([WaT, queryT[:, bs]], axis=1).reshape(
            HC, 128, H + BL
        )
        wqpk = np.ascontiguousarray(wqpk.transpose(1, 0, 2))
        in_maps.append(
            {
                "keysT": np.ascontiguousarray(keysT[bs]),
                "uapack": uapk,
                "wqpack": wqpk,
                "bias_wu": bias_wu,
                "va": va_f,
                "vab": vab,
                "ident": ident,
            }
        )
    return in_maps


def kernel(query, keys, Wa_w, Wa_b, Ua_w, Ua_b, Va_w, Va_b):
    from concourse.bass_utils import run_bass_kernel_spmd

    query, keys = np.asarray(query), np.asarray(keys)
    Wa_w, Wa_b = np.asarray(Wa_w), np.asarray(Wa_b)
    Ua_w, Ua_b = np.asarray(Ua_w), np.asarray(Ua_b)
    Va_w, Va_b = np.asarray(Va_w), np.asarray(Va_b)
    if "nc" not in _cache:
        _cache["nc"] = _build()
    nc = _cache["nc"]

    in_maps = _prep_in_maps(query, keys, Wa_w, Wa_b, Ua_w, Ua_b, Va_w, Va_b)
    res = run_bass_kernel_spmd(nc, in_maps, core_ids=list(range(NCORES)))
    context = np.concatenate(
        [res.results[c]["out_ctx"] for c in range(NCORES)], axis=0
    )
    attn = np.concatenate(
        [res.results[c]["out_attn"] for c in range(NCORES)], axis=0
    )
    return context, attn


# revision 43
# speedup vs baseline: 1.1987x; 1.0032x over previous
"""Bahdanau additive attention on 8 Trainium2 NeuronCores.

reference:
    q_proj = (query @ Wa_w.T + Wa_b)[:, None, :]          # [B, 1, H]
    k_proj = einsum('bsh,dh->bsd', keys, Ua_w) + Ua_b     # [B, S, H]
    scores = einsum('bsh,h->bs', tanh(q_proj + k_proj), Va_w[0]) + Va_b[0]
    attn   = softmax(scores, axis=1)                      # [B, S]
    context= einsum('bs,bsh->bh', attn, keys)             # [B, H]

Sharding: pure data parallel over B (64 -> 8 per core); weights replicated.

Device pipeline (per core, B_loc=8, S=2048, H=D=512), all matmuls bf16:
  - keysT bf16 (h on partitions) drives k_projT = Ua_w @ k^T in PSUM;
    tanh + per-partition q_proj bias fused on ScalarE.
  - scores row via M=1 matmuls (lhsT = Va column) accumulated in PSUM.
  - p = exp(scores + Va_b) on ScalarE with fused running sum (accum_out).
    No max-subtraction: |scores| <= sum|Va| + |Va_b| ~ 23 fits fp32 exp.
  - context rides VectorE: one K=1 matmul broadcasts the p chunk across
    partitions, then scalar_tensor_tensor (float scalar + fused accum_out)
    does multiply+free-reduce against the resident keysT tiles in a single
    DVE op per h-tile — no second keys copy, no p transposes.
  - per batch the four [128,1] context columns are transposed back to one
    [1,512] PSUM row with identity matmuls and scaled by 1/l in one op.
  - Three-stage software pipeline over (batch, s-chunk) groups g:
    projection matmuls of g, score matmuls of g-1, context of g-2 — every
    PE instruction's ACT/DVE dependency is a full chunk old, so the PE FIFO
    streams at the N=512 back-to-back rate (~216 ns/matmul).
"""

import numpy as np
import ml_dtypes

B, S, H = 64, 2048, 512
NCORES = 8
BL = B // NCORES          # batches per core
HC = H // 128             # h chunks (contraction)
DT = H // 128             # d tiles (partition dim of k_projT)
SCW = 512                 # s-chunk width (PSUM bank limit)
SC = S // SCW             # s chunks
ST = S // 128             # s tiles (for p^T / context)

_cache = {}


def _build(do_compile=True):
    import concourse.bass as bass
    import concourse.tile as tile
    from concourse import bacc, mybir

    f32 = mybir.dt.float32
    bf16 = mybir.dt.bfloat16
    AF = mybir.ActivationFunctionType
    ALU = mybir.AluOpType
    AX = mybir.AxisListType

    nc = bacc.Bacc("TRN2", target_bir_lowering=False, debug=False)

    keysT = nc.declare_dram_parameter("keysT", [BL, H, S], bf16, isOutput=False)
    uapack = nc.declare_dram_parameter(
        "uapack", [128, HC, H], bf16, isOutput=False
    )
    wqpack = nc.declare_dram_parameter(
        "wqpack", [128, HC, H + BL], bf16, isOutput=False
    )
    bias_wu = nc.declare_dram_parameter("bias_wu", [128, DT], f32, isOutput=False)
    va = nc.declare_dram_parameter("va", [128, DT], bf16, isOutput=False)
    vab = nc.declare_dram_parameter("vab", [1, 1], f32, isOutput=False)
    ident = nc.declare_dram_parameter("ident", [128, 128], f32, isOutput=False)
    out_ctx = nc.declare_dram_parameter("out_ctx", [BL, H], f32, isOutput=True)
    out_attn = nc.declare_dram_parameter("out_attn", [BL, S], f32, isOutput=True)

    with tile.TileContext(nc) as tc:
        with (
            tc.tile_pool(name="persist", bufs=1) as pp,
            tc.tile_pool(name="keys", bufs=2) as kp,
            tc.tile_pool(name="work", bufs=10) as wp,
            tc.tile_pool(name="rows", bufs=3) as rp,
            tc.tile_pool(name="outs", bufs=2) as op,
            tc.tile_pool(name="ps", bufs=1, space="PSUM") as ps,
        ):
            ua_sb = pp.tile([128, HC, H], bf16, tag="uapack")
            UaT_sb = [ua_sb[:, hc, :] for hc in range(HC)]
            bwu_sb = pp.tile([128, DT], f32, tag="bwu")
            nc.gpsimd.dma_start(bwu_sb[:], bias_wu[:])
            va_sb = pp.tile([128, DT], bf16, tag="va")
            nc.gpsimd.dma_start(va_sb[:], va[:])
            vab_sb = pp.tile([1, 1], f32, tag="vab")
            nc.gpsimd.dma_start(vab_sb[:], vab[:])
            ones_bf = pp.tile([1, 128], bf16, tag="ones_bf")
            nc.vector.memset(ones_bf[:], 1.0)
            ident_sb = pp.tile([128, 128], f32, tag="ident")
            nc.gpsimd.dma_start(ident_sb[:], ident[:])

            qb_sb = []

            def q_proj():
                for dt in range(DT):
                    pq = ps.tile([128, BL], f32, tag="pmisc", name="pq", bufs=1)
                    dsl = slice(128 * dt, 128 * (dt + 1))
                    for hc in range(HC):
                        nc.tensor.matmul(
                            pq[:], wq_sb[:, hc, dsl], wq_sb[:, hc, H : H + BL],
                            start=(hc == 0), stop=(hc == HC - 1),
                        )
                    qb = pp.tile([128, BL], f32, tag=f"qb{dt}", name=f"qb{dt}")
                    nc.scalar.activation(
                        qb[:], pq[:], AF.Identity, bias=bwu_sb[:, dt : dt + 1]
                    )
                    qb_sb.append(qb)

            l_parts = pp.tile([1, BL * SC], f32, tag="l_parts")

            # per-batch live state
            kT = [None] * BL
            p_row = [None] * BL

            wq_sb = pp.tile([128, HC, H + BL], bf16, tag="wqpack")

            def load_batch(b, interleave_ua=False):
                kT[b] = []
                if interleave_ua:
                    nc.sync.dma_start(ua_sb[:], uapack[:])
                for ht in range(HC):
                    t = kp.tile([128, S], bf16, tag=f"kT{ht}", name=f"kT{ht}_{b}", bufs=3)
                    nc.sync.dma_start(
                        t[:], keysT[b, 128 * ht : 128 * (ht + 1), :]
                    )
                    kT[b].append(t)
                    if interleave_ua and ht == 0:
                        nc.sync.dma_start(wq_sb[:], wqpack[:])
                p_row[b] = rp.tile([1, S], bf16, tag="p_row", name=f"p_row{b}")

            state = {}

            def proj_chunk(b, sc, mid_hook=None, startup=False):
                """16 projection MMs + 4 tanh; score MMs run one chunk later.

                startup: h-chunk-major order over the first 3 banks so the
                first matmuls only need the first keysT tile's DMA."""
                ssl = slice(SCW * sc, SCW * (sc + 1))
                pks = [
                    ps.tile([128, SCW], f32, tag="pk", name="pk", bufs=3)
                    for _ in range(DT)
                ]
                order = [(hc, dt) for dt in range(DT) for hc in range(HC)]
                for hc, dt in order:
                    dsl = slice(128 * dt, 128 * (dt + 1))
                    nc.tensor.matmul(
                        pks[dt][:], UaT_sb[hc][:, dsl], kT[b][hc][:, ssl],
                        start=(hc == 0), stop=(hc == HC - 1),
                    )
                if mid_hook is not None:
                    mid_hook()
                t_tiles = []
                for dt in range(DT):
                    t_sb = wp.tile([128, SCW], bf16, tag="t", name="t")
                    nc.scalar.activation(
                        t_sb[:], pks[dt][:], AF.Tanh, bias=qb_sb[dt][:, b : b + 1]
                    )
                    t_tiles.append(t_sb)
                state[(b, sc)] = t_tiles

            def score_chunk(b, sc):
                """4 score MMs (tanh results are a chunk old -> no PE stall),
                then exp with fused partial sum."""
                ssl = slice(SCW * sc, SCW * (sc + 1))
                t_tiles = state.pop((b, sc))
                psc = ps.tile([1, SCW], f32, tag="psc", name=f"psc{b}_{sc}", bufs=2)
                for dt in range(DT):
                    nc.tensor.matmul(
                        psc[:], va_sb[:, dt : dt + 1], t_tiles[dt][:],
                        start=(dt == 0), stop=(dt == DT - 1),
                    )
                col = SC * b + sc
                nc.scalar.activation(
                    p_row[b][0:1, ssl], psc[:], AF.Exp, bias=vab_sb[0:1, 0:1],
                    accum_out=l_parts[0:1, col : col + 1],
                )
                pb = ps.tile([128, SCW], f32, tag="pb", name="pb", bufs=2)
                nc.tensor.matmul(
                    pb[:], ones_bf[:], p_row[b][0:1, ssl], start=True, stop=True
                )
                pb_state[(b, sc)] = pb

            pb_state = {}
            ctx_parts = [
                pp.tile([128, BL * SC], f32, tag=f"ctxp{ht}", name=f"ctxp{ht}")
                for ht in range(HC)
            ]

            def ctx_chunk(b, sc):
                """Broadcast the p chunk across partitions with one K=1 matmul,
                then fused multiply+free-reduce on VectorE per keysT h-tile:
                ctx_parts[ht][:, col] = sum_s kT[ht][:, s] * p[s]."""
                ssl = slice(SCW * sc, SCW * (sc + 1))
                col = SC * b + sc
                pb = pb_state.pop((b, sc))
                for ht in range(HC):
                    scr = wp.tile([128, SCW], bf16, tag="scr", name="scr", bufs=4)
                    nc.vector.scalar_tensor_tensor(
                        scr[:], kT[b][ht][:, ssl], 1.0, pb[:],
                        ALU.mult, ALU.mult,
                        accum_out=ctx_parts[ht][:, col : col + 1],
                    )

            def finish_batch(b):
                """softmax normalization + outputs for batch b."""
                lsum = rp.tile([1, 1], f32, tag="lsum", name=f"lsum{b}")
                nc.vector.tensor_reduce(
                    lsum[:], l_parts[0:1, SC * b : SC * (b + 1)], AX.X, ALU.add
                )
                linv_b = rp.tile([1, 1], f32, tag="linv", name=f"linv{b}")
                nc.vector.reciprocal(linv_b[:], lsum[:])
                attn_row = op.tile([1, S], f32, tag="attn_row", name="attn_row")
                nc.vector.tensor_scalar_mul(attn_row[:], p_row[b][:], linv_b[:])
                dma_eng = nc.sync if b == BL - 1 else nc.gpsimd
                dma_eng.dma_start(out_attn[b : b + 1, :], attn_row[:])

                # transpose the 4 context columns back to one [1, 512] row
                # via K=128/M=1/N=128 identity matmuls, then scale by 1/l
                pcr = ps.tile([1, H], f32, tag="pmisc", name=f"pcr{b}", bufs=1)
                for ht in range(HC):
                    csum = rp.tile([128, 1], f32, tag="csum", name=f"csum{b}_{ht}")
                    nc.vector.tensor_reduce(
                        csum[:], ctx_parts[ht][:, SC * b : SC * (b + 1)],
                        AX.X, ALU.add,
                    )
                    nc.tensor.matmul(
                        pcr[0:1, 128 * ht : 128 * (ht + 1)], csum[:], ident_sb[:],
                        start=True, stop=True,
                    )
                ctx_row = op.tile([1, H], f32, tag="ctx_row", name="ctx_row")
                nc.vector.tensor_scalar_mul(ctx_row[:], pcr[:], linv_b[:])
                dma_eng.dma_start(out_ctx[b : b + 1, :], ctx_row[:])

            # ---- software-pipelined emission over (batch, chunk) ----
            load_batch(0, interleave_ua=True)
            chunks = [(b, sc) for b in range(BL) for sc in range(SC)]
            N = len(chunks)
            for g in range(N + 2):
                if g < N:
                    b, sc = chunks[g]
                    if sc == 0 and b + 1 < BL:
                        load_batch(b + 1)   # prefetch next batch's keys
                    proj_chunk(b, sc, mid_hook=(q_proj if g == 0 else None), startup=(g == 0))
                if 0 <= g - 1 < N:
                    score_chunk(*chunks[g - 1])
                if 0 <= g - 2 < N:
                    tb, tsc = chunks[g - 2]
                    ctx_chunk(tb, tsc)
                    if tsc == SC - 1:
                        finish_batch(tb)

    if do_compile:
        nc.compile()
    return nc


def _prep_in_maps(query, keys, Wa_w, Wa_b, Ua_w, Ua_b, Va_w, Va_b):
    bf16 = ml_dtypes.bfloat16
    keysT = np.ascontiguousarray(keys.astype(bf16).transpose(0, 2, 1))  # [B,H,S]
    queryT = query.T.astype(bf16)                               # [H, B]
    WaT = Wa_w.T.astype(bf16)
    UaT = Ua_w.T.astype(bf16)
    bias_wu = np.ascontiguousarray(
        (Wa_b + Ua_b).astype(np.float32).reshape(DT, 128).T
    )
    va_f = np.ascontiguousarray(Va_w[0].astype(bf16).reshape(DT, 128).T)
    vab = Va_b.reshape(1, 1).astype(np.float32)
    ident = np.eye(128, dtype=np.float32)

    in_maps = []
    for c in range(NCORES):
        bs = slice(BL * c, BL * (c + 1))
        uapk = np.ascontiguousarray(
            UaT.reshape(HC, 128, H).transpose(1, 0, 2)
        )
        wqpk = np.concatenate([WaT, queryT[:, bs]], axis=1).reshape(
            HC, 128, H + BL
        )
        wqpk = np.ascontiguousarray(wqpk.transpose(1, 0, 2))
        in_maps.append(
            {
                "keysT": np.ascontiguousarray(keysT[bs]),
                "uapack": uapk,
                "wqpack": wqpk,
                "bias_wu": bias_wu,
                "va": va_f,
                "vab": vab,
                "ident": ident,
            }
        )
    return in_maps


def kernel(query, keys, Wa_w, Wa_b, Ua_w, Ua_b, Va_w, Va_b):
    from concourse.bass_utils import run_bass_kernel_spmd

    query, keys = np.asarray(query), np.asarray(keys)
    Wa_w, Wa_b = np.asarray(Wa_w), np.asarray(Wa_b)
    Ua_w, Ua_b = np.asarray(Ua_w), np.asarray(Ua_b)
    Va_w, Va_b = np.asarray(Va_w), np.asarray(Va_b)
    if "nc" not in _cache:
        _cache["nc"] = _build()
    nc = _cache["nc"]

    in_maps = _prep_in_maps(query, keys, Wa_w, Wa_b, Ua_w, Ua_b, Va_w, Va_b)
    res = run_bass_kernel_spmd(nc, in_maps, core_ids=list(range(NCORES)))
    context = np.concatenate(
        [res.results[c]["out_ctx"] for c in range(NCORES)], axis=0
    )
    attn = np.concatenate(
        [res.results[c]["out_attn"] for c in range(NCORES)], axis=0
    )
    return context, attn


# revision 44
# speedup vs baseline: 1.2095x; 1.0091x over previous
"""Bahdanau additive attention on 8 Trainium2 NeuronCores.

reference:
    q_proj = (query @ Wa_w.T + Wa_b)[:, None, :]          # [B, 1, H]
    k_proj = einsum('bsh,dh->bsd', keys, Ua_w) + Ua_b     # [B, S, H]
    scores = einsum('bsh,h->bs', tanh(q_proj + k_proj), Va_w[0]) + Va_b[0]
    attn   = softmax(scores, axis=1)                      # [B, S]
    context= einsum('bs,bsh->bh', attn, keys)             # [B, H]

Sharding: pure data parallel over B (64 -> 8 per core); weights replicated.

Device pipeline (per core, B_loc=8, S=2048, H=D=512), all matmuls bf16:
  - keysT bf16 (h on partitions) drives k_projT = Ua_w @ k^T in PSUM;
    tanh + per-partition q_proj bias fused on ScalarE.
  - scores row via M=1 matmuls (lhsT = Va column) accumulated in PSUM.
  - p = exp(scores + Va_b) on ScalarE with fused running sum (accum_out).
    No max-subtraction: |scores| <= sum|Va| + |Va_b| ~ 23 fits fp32 exp.
  - context rides VectorE: one K=1 matmul broadcasts the p chunk across
    partitions, then scalar_tensor_tensor (float scalar + fused accum_out)
    does multiply+free-reduce against the resident keysT tiles in a single
    DVE op per h-tile — no second keys copy, no p transposes.
  - per batch the four [128,1] context columns are transposed back to one
    [1,512] PSUM row with identity matmuls and scaled by 1/l in one op.
  - Three-stage software pipeline over (batch, s-chunk) groups g:
    projection matmuls of g, score matmuls of g-1, context of g-2 — every
    PE instruction's ACT/DVE dependency is a full chunk old, so the PE FIFO
    streams at the N=512 back-to-back rate (~216 ns/matmul).
"""

import numpy as np
import ml_dtypes

B, S, H = 64, 2048, 512
NCORES = 8
BL = B // NCORES          # batches per core
HC = H // 128             # h chunks (contraction)
DT = H // 128             # d tiles (partition dim of k_projT)
SCW = 512                 # s-chunk width (PSUM bank limit)
SC = S // SCW             # s chunks
ST = S // 128             # s tiles (for p^T / context)

_cache = {}


def _build(do_compile=True):
    import concourse.bass as bass
    import concourse.tile as tile
    from concourse import bacc, mybir

    f32 = mybir.dt.float32
    bf16 = mybir.dt.bfloat16
    AF = mybir.ActivationFunctionType
    ALU = mybir.AluOpType
    AX = mybir.AxisListType

    nc = bacc.Bacc("TRN2", target_bir_lowering=False, debug=False)

    keysT = nc.declare_dram_parameter("keysT", [BL, H, S], bf16, isOutput=False)
    uapack = nc.declare_dram_parameter(
        "uapack", [128, HC, H], bf16, isOutput=False
    )
    wqpack = nc.declare_dram_parameter(
        "wqpack", [128, HC, H + BL], bf16, isOutput=False
    )
    bias_wu = nc.declare_dram_parameter("bias_wu", [128, DT], f32, isOutput=False)
    va = nc.declare_dram_parameter("va", [128, DT], bf16, isOutput=False)
    vab = nc.declare_dram_parameter("vab", [1, 1], f32, isOutput=False)
    ident = nc.declare_dram_parameter("ident", [128, 128], f32, isOutput=False)
    out_ctx = nc.declare_dram_parameter("out_ctx", [BL, H], f32, isOutput=True)
    out_attn = nc.declare_dram_parameter("out_attn", [BL, S], f32, isOutput=True)

    with tile.TileContext(nc) as tc:
        with (
            tc.tile_pool(name="persist", bufs=1) as pp,
            tc.tile_pool(name="keys", bufs=2) as kp,
            tc.tile_pool(name="work", bufs=10) as wp,
            tc.tile_pool(name="rows", bufs=3) as rp,
            tc.tile_pool(name="outs", bufs=2) as op,
            tc.tile_pool(name="ps", bufs=1, space="PSUM") as ps,
        ):
            ua_sb = pp.tile([128, HC, H], bf16, tag="uapack")
            UaT_sb = [ua_sb[:, hc, :] for hc in range(HC)]
            bwu_sb = pp.tile([128, DT], f32, tag="bwu")
            nc.gpsimd.dma_start(bwu_sb[:], bias_wu[:])
            va_sb = pp.tile([128, DT], bf16, tag="va")
            nc.gpsimd.dma_start(va_sb[:], va[:])
            vab_sb = pp.tile([1, 1], f32, tag="vab")
            nc.gpsimd.dma_start(vab_sb[:], vab[:])
            ones_bf = pp.tile([1, 128], bf16, tag="ones_bf")
            nc.vector.memset(ones_bf[:], 1.0)
            ident_sb = pp.tile([128, 128], f32, tag="ident")
            nc.gpsimd.dma_start(ident_sb[:], ident[:])

            qb_sb = []

            def q_proj():
                for dt in range(DT):
                    pq = ps.tile([128, BL], f32, tag="pmisc", name="pq", bufs=1)
                    dsl = slice(128 * dt, 128 * (dt + 1))
                    for hc in range(HC):
                        nc.tensor.matmul(
                            pq[:], wq_sb[:, hc, dsl], wq_sb[:, hc, H : H + BL],
                            start=(hc == 0), stop=(hc == HC - 1),
                        )
                    qb = pp.tile([128, BL], f32, tag=f"qb{dt}", name=f"qb{dt}")
                    nc.scalar.activation(
                        qb[:], pq[:], AF.Identity, bias=bwu_sb[:, dt : dt + 1]
                    )
                    qb_sb.append(qb)

            l_parts = pp.tile([1, BL * SC], f32, tag="l_parts")

            # per-batch live state
            kT = [None] * BL
            p_row = [None] * BL

            wq_sb = pp.tile([128, HC, H + BL], bf16, tag="wqpack")

            def load_batch(b, interleave_ua=False):
                kT[b] = []
                if interleave_ua:
                    nc.sync.dma_start(ua_sb[:], uapack[:])
                for ht in range(HC):
                    t = kp.tile([128, S], bf16, tag=f"kT{ht}", name=f"kT{ht}_{b}", bufs=3)
                    nc.sync.dma_start(
                        t[:], keysT[b, 128 * ht : 128 * (ht + 1), :]
                    )
                    kT[b].append(t)
                    if interleave_ua and ht == 0:
                        nc.sync.dma_start(wq_sb[:], wqpack[:])
                p_row[b] = rp.tile([1, S], bf16, tag="p_row", name=f"p_row{b}")

            state = {}

            def proj_chunk(b, sc, mid_hook=None, startup=False):
                """16 projection MMs + 4 tanh; score MMs run one chunk later.

                startup: h-chunk-major order over the first 3 banks so the
                first matmuls only need the first keysT tile's DMA."""
                ssl = slice(SCW * sc, SCW * (sc + 1))
                pks = [
                    ps.tile([128, SCW], f32, tag="pk", name="pk", bufs=4)
                    for _ in range(DT)
                ]
                order = [(hc, dt) for dt in range(DT) for hc in range(HC)]
                for hc, dt in order:
                    dsl = slice(128 * dt, 128 * (dt + 1))
                    nc.tensor.matmul(
                        pks[dt][:], UaT_sb[hc][:, dsl], kT[b][hc][:, ssl],
                        start=(hc == 0), stop=(hc == HC - 1),
                    )
                if mid_hook is not None:
                    mid_hook()
                t_tiles = []
                for dt in range(DT):
                    t_sb = wp.tile([128, SCW], bf16, tag="t", name="t")
                    nc.scalar.activation(
                        t_sb[:], pks[dt][:], AF.Tanh, bias=qb_sb[dt][:, b : b + 1]
                    )
                    t_tiles.append(t_sb)
                state[(b, sc)] = t_tiles

            def score_chunk(b, sc):
                """4 score MMs (tanh results are a chunk old -> no PE stall),
                then exp with fused partial sum."""
                ssl = slice(SCW * sc, SCW * (sc + 1))
                t_tiles = state.pop((b, sc))
                psc = ps.tile([1, SCW], f32, tag="psc", name=f"psc{b}_{sc}", bufs=1)
                for dt in range(DT):
                    nc.tensor.matmul(
                        psc[:], va_sb[:, dt : dt + 1], t_tiles[dt][:],
                        start=(dt == 0), stop=(dt == DT - 1),
                    )
                col = SC * b + sc
                nc.scalar.activation(
                    p_row[b][0:1, ssl], psc[:], AF.Exp, bias=vab_sb[0:1, 0:1],
                    accum_out=l_parts[0:1, col : col + 1],
                )
                pb = ps.tile([128, SCW], f32, tag="pb", name="pb", bufs=2)
                nc.tensor.matmul(
                    pb[:], ones_bf[:], p_row[b][0:1, ssl], start=True, stop=True
                )
                pb_state[(b, sc)] = pb

            pb_state = {}
            ctx_parts = [
                pp.tile([128, BL * SC], f32, tag=f"ctxp{ht}", name=f"ctxp{ht}")
                for ht in range(HC)
            ]

            def ctx_chunk(b, sc):
                """Broadcast the p chunk across partitions with one K=1 matmul,
                then fused multiply+free-reduce on VectorE per keysT h-tile:
                ctx_parts[ht][:, col] = sum_s kT[ht][:, s] * p[s]."""
                ssl = slice(SCW * sc, SCW * (sc + 1))
                col = SC * b + sc
                pb = pb_state.pop((b, sc))
                for ht in range(HC):
                    scr = wp.tile([128, SCW], bf16, tag="scr", name="scr", bufs=4)
                    nc.vector.scalar_tensor_tensor(
                        scr[:], kT[b][ht][:, ssl], 1.0, pb[:],
                        ALU.mult, ALU.mult,
                        accum_out=ctx_parts[ht][:, col : col + 1],
                    )

            def finish_batch(b):
                """softmax normalization + outputs for batch b."""
                lsum = rp.tile([1, 1], f32, tag="lsum", name=f"lsum{b}")
                nc.vector.tensor_reduce(
                    lsum[:], l_parts[0:1, SC * b : SC * (b + 1)], AX.X, ALU.add
                )
                linv_b = rp.tile([1, 1], f32, tag="linv", name=f"linv{b}")
                nc.vector.reciprocal(linv_b[:], lsum[:])
                attn_row = op.tile([1, S], f32, tag="attn_row", name="attn_row")
                nc.vector.tensor_scalar_mul(attn_row[:], p_row[b][:], linv_b[:])
                dma_eng = nc.sync if b == BL - 1 else nc.gpsimd
                dma_eng.dma_start(out_attn[b : b + 1, :], attn_row[:])

                # transpose the 4 context columns back to one [1, 512] row
                # via K=128/M=1/N=128 identity matmuls, then scale by 1/l
                pcr = ps.tile([1, H], f32, tag="pmisc", name=f"pcr{b}", bufs=1)
                for ht in range(HC):
                    csum = rp.tile([128, 1], f32, tag="csum", name=f"csum{b}_{ht}")
                    nc.vector.tensor_reduce(
                        csum[:], ctx_parts[ht][:, SC * b : SC * (b + 1)],
                        AX.X, ALU.add,
                    )
                    nc.tensor.matmul(
                        pcr[0:1, 128 * ht : 128 * (ht + 1)], csum[:], ident_sb[:],
                        start=True, stop=True,
                    )
                ctx_row = op.tile([1, H], f32, tag="ctx_row", name="ctx_row")
                nc.vector.tensor_scalar_mul(ctx_row[:], pcr[:], linv_b[:])
                dma_eng.dma_start(out_ctx[b : b + 1, :], ctx_row[:])

            # ---- software-pipelined emission over (batch, chunk) ----
            load_batch(0, interleave_ua=True)
            chunks = [(b, sc) for b in range(BL) for sc in range(SC)]
            N = len(chunks)
            for g in range(N + 2):
                if g < N:
                    b, sc = chunks[g]
                    if sc == 0 and b + 1 < BL:
                        load_batch(b + 1)   # prefetch next batch's keys
                    proj_chunk(b, sc, mid_hook=(q_proj if g == 0 else None), startup=(g == 0))
                if 0 <= g - 1 < N:
                    score_chunk(*chunks[g - 1])
                if 0 <= g - 2 < N:
                    tb, tsc = chunks[g - 2]
                    ctx_chunk(tb, tsc)
                    if tsc == SC - 1:
                        finish_batch(tb)

    if do_compile:
        nc.compile()
    return nc


def _prep_in_maps(query, keys, Wa_w, Wa_b, Ua_w, Ua_b, Va_w, Va_b):
    bf16 = ml_dtypes.bfloat16
    keysT = np.ascontiguousarray(keys.astype(bf16).transpose(0, 2, 1))  # [B,H,S]
    queryT = query.T.astype(bf16)                               # [H, B]
    WaT = Wa_w.T.astype(bf16)
    UaT = Ua_w.T.astype(bf16)
    bias_wu = np.ascontiguousarray(
        (Wa_b + Ua_b).astype(np.float32).reshape(DT, 128).T
    )
    va_f = np.ascontiguousarray(Va_w[0].astype(bf16).reshape(DT, 128).T)
    vab = Va_b.reshape(1, 1).astype(np.float32)
    ident = np.eye(128, dtype=np.float32)

    in_maps = []
    for c in range(NCORES):
        bs = slice(BL * c, BL * (c + 1))
        uapk = np.ascontiguousarray(
            UaT.reshape(HC, 128, H).transpose(1, 0, 2)
        )
        wqpk = np.concatenate([WaT, queryT[:, bs]], axis=1).reshape(
            HC, 128, H + BL
        )
        wqpk = np.ascontiguousarray(wqpk.transpose(1, 0, 2))
        in_maps.append(
            {
                "keysT": np.ascontiguousarray(keysT[bs]),
                "uapack": uapk,
                "wqpack": wqpk,
                "bias_wu": bias_wu,
                "va": va_f,
                "vab": vab,
                "ident": ident,
            }
        )
    return in_maps


def kernel(query, keys, Wa_w, Wa_b, Ua_w, Ua_b, Va_w, Va_b):
    from concourse.bass_utils import run_bass_kernel_spmd

    query, keys = np.asarray(query), np.asarray(keys)
    Wa_w, Wa_b = np.asarray(Wa_w), np.asarray(Wa_b)
    Ua_w, Ua_b = np.asarray(Ua_w), np.asarray(Ua_b)
    Va_w, Va_b = np.asarray(Va_w), np.asarray(Va_b)
    if "nc" not in _cache:
        _cache["nc"] = _build()
    nc = _cache["nc"]

    in_maps = _prep_in_maps(query, keys, Wa_w, Wa_b, Ua_w, Ua_b, Va_w, Va_b)
    res = run_bass_kernel_spmd(nc, in_maps, core_ids=list(range(NCORES)))
    context = np.concatenate(
        [res.results[c]["out_ctx"] for c in range(NCORES)], axis=0
    )
    attn = np.concatenate(
        [res.results[c]["out_attn"] for c in range(NCORES)], axis=0
    )
    return context, attn
